# revision 42
# baseline (speedup 1.0000x reference)
"""Trainium2 Bass kernel for EquivariantUpsampleConv (H=W=256, R=4, C=22).

Strategy: data-parallel over LR rows (32 per core, 8 cores), with a 1-row
halo. Exploits the x4 nearest-neighbour upsample structure: within each
4x4 HR block there are only 4 distinct outputs (interior / right-edge /
bottom-edge / corner), so all compute runs at LR resolution (65536 px)
instead of HR (1048576 px); the 4x4 replication happens in the output DMA.

Per-pixel math: ctx_v = sum_u F_u @ M_{u,v} (stage-1, fp32r matmuls over
shifted channel-major feature views); out_v[c] = sum_{a,b} F00[a] ctx_v[b]
T[a,b,c] + F00[c]. Stage-2 per 128-px chunk: one fp32r matmul with
lhsT=ctx chunk gives G2[px,(c,a)] = sum_b T[a,b,c] ctx[b]; a custom DVE
op (MUL_SCAN_ANT) computes the running prefix sum of G2*broadcast(F00)
in one pass, and per-c segment sums are recovered as strided differences
of the prefix (fp32, stable). The residual is added in fp32 by the
epilogue block-replication adds.
"""

import math
import re
import sys
import types

import numpy as np

if "/opt/trn_rl_repo" not in sys.path:
    sys.path.insert(0, "/opt/trn_rl_repo")

import concourse.bacc as bacc
import concourse.dve_ops as dve_ops
import concourse.mybir as mybir
from concourse import tile
from concourse.bass_utils import run_bass_kernel_spmd
from concourse.dve_ops import DveOp
from concourse.dve_spec import AluOp, Scan, Spec, Src0, Src1

# ---------------------------------------------------------------------------
# Problem constants (hardcoded per spec)
# ---------------------------------------------------------------------------
H = W = 256
R = 4
C_FEAT = 22
N_CORES = 8
ROWS_PER_CORE = H // N_CORES          # 32 LR rows
LR_PX_PER_CORE = ROWS_PER_CORE * W    # 8192
WP = W + 1                            # 257: x-padded row stride
N_TILES = 16                          # 2 LR rows (512 px) per tile
TILE_PX = 512
N_CHUNKS = 4                          # 128-px chunks per tile

FEAT_SLICES = {4: (0, 9), 6: (9, 22)}
SH_SLICES = {0: (0, 1), 2: (1, 6)}
AGG_PATHS = [(4, 0, 4), (4, 2, 4), (4, 2, 6), (6, 0, 6), (6, 2, 4), (6, 2, 6)]
TP_PATHS = [(4, 4, 4), (4, 4, 6), (4, 6, 4), (4, 6, 6),
            (6, 4, 4), (6, 4, 6), (6, 6, 4), (6, 6, 6)]
AGG_NPATH = {4: 3, 6: 3}
TP_NPATH = {4: 4, 6: 4}

f32 = mybir.dt.float32


# ---------------------------------------------------------------------------
# Custom DVE op: prefix-scan of Src0*Src1 along the free stream
# ---------------------------------------------------------------------------
def _make_mul_scan():
    for o in dve_ops.OPS:
        if o.name == "MUL_SCAN_ANT":
            return o

    def _ref(in0, in1, c0, c1, c2):
        sh = in0.shape
        a = np.asarray(in0, np.float32).reshape(sh[0], -1)
        b = np.asarray(in1, np.float32).reshape(sh[0], -1)
        return np.add.accumulate(a * b, axis=1, dtype=np.float32).reshape(sh)

    spec = Spec(body=Scan(AluOp.ADD, Src0 * Src1), reference=_ref)
    dve_ops._SUB_OPCODE_FOR_NAME["MUL_SCAN_ANT"] = (
        dve_ops._CUSTOM_DVE_ROW_BASE + len(dve_ops.OPS))
    op = DveOp("MUL_SCAN_ANT", spec, subdim=False, uops_sha={})
    for ver in ("v3", "v4"):
        try:
            op.compile(ver)
        except ValueError as e:
            m = re.search(r'="([0-9a-f]+)"', str(e))
            assert m, str(e)
            op.uops_sha[ver] = m.group(1)
            dve_ops._COMPILE_CACHE.pop((op.name, ver), None)
    op.compile("v3")
    dve_ops.OPS.append(op)
    dve_ops.CUSTOM_DVE_SPECS[op.name] = spec
    return op


MUL_SCAN = _make_mul_scan()


# ---------------------------------------------------------------------------
# Clebsch-Gordan / spherical-harmonic constants (pure math, float64)
# ---------------------------------------------------------------------------
def _cg(j1, m1, j2, m2, j3, m3):
    if m1 + m2 != m3:
        return 0.0
    f = math.factorial
    pre = (2 * j3 + 1) * f(j1 + j2 - j3) * f(j1 - j2 + j3) * f(-j1 + j2 + j3) / f(j1 + j2 + j3 + 1)
    pre *= f(j1 + m1) * f(j1 - m1) * f(j2 + m2) * f(j2 - m2) * f(j3 + m3) * f(j3 - m3)
    kmin = max(0, j2 - j3 - m1, j1 - j3 + m2)
    kmax = min(j1 + j2 - j3, j1 - m1, j2 + m2)
    s = 0.0
    for k in range(kmin, kmax + 1):
        s += (-1) ** k / (f(k) * f(j1 + j2 - j3 - k) * f(j1 - m1 - k) * f(j2 + m2 - k)
                          * f(j3 - j2 + m1 + k) * f(j3 - j1 - m2 + k))
    return math.sqrt(pre) * s


def _q(l):
    q = np.zeros((2 * l + 1, 2 * l + 1), dtype=np.complex128)
    q[l, l] = 1.0
    s2 = 1.0 / math.sqrt(2.0)
    for m in range(1, l + 1):
        q[l + m, l + m] = (-1) ** m * s2
        q[l + m, l - m] = s2
        q[l - m, l - m] = 1j * s2
        q[l - m, l + m] = -1j * (-1) ** m * s2
    return ((-1j) ** l) * q


def _w3j_real(l1, l2, l3):
    w = np.zeros((2 * l1 + 1, 2 * l2 + 1, 2 * l3 + 1))
    for m1 in range(-l1, l1 + 1):
        for m2 in range(-l2, l2 + 1):
            m3 = -(m1 + m2)
            if abs(m3) <= l3:
                w[l1 + m1, l2 + m2, l3 + m3] = ((-1) ** (l1 - l2 - m3)) / math.sqrt(2 * l3 + 1) * _cg(l1, m1, l2, m2, l3, -m3)
    c = np.einsum('ai,bj,ck,ijk->abc', _q(l1), _q(l2), _q(l3), w.astype(np.complex128))
    return np.real(c)


_CG_T = {p: _w3j_real(*p) for p in set(AGG_PATHS) | set(TP_PATHS)}


def _sh_kernel():
    s = 1.0 / math.sqrt(2.0)
    dirs = np.array([[-s, -s, 0.0], [s, -s, 0.0], [-s, s, 0.0], [s, s, 0.0]])
    x, y, z = dirs[:, 0], dirs[:, 1], dirs[:, 2]
    c2 = math.sqrt(15.0 / math.pi)
    return np.stack([
        np.full(4, 0.5 / math.sqrt(math.pi)),
        0.5 * c2 * x * y,
        0.5 * c2 * y * z,
        0.25 * math.sqrt(5.0 / math.pi) * (3 * z ** 2 - 1.0),
        0.5 * c2 * x * z,
        0.25 * c2 * (x ** 2 - y ** 2),
    ], axis=1)  # (4, 6)


_SH = _sh_kernel()


def _fold_weights(w_agg, w_tp):
    """Fold runtime path weights into device matrices.

    Returns:
      s1w [9, 22, 22] float32: stage-1 lhsT mats (a-in -> c-out), order
        [S_int, S_A, S_B, S_C, S_D, M0, M1, M2, M3].
      t_unf [22, 484] float32: T[a,b,c] unfolded, col = c*22 + a, row b.
    """
    w_agg = np.asarray(w_agg, dtype=np.float64)
    w_tp = np.asarray(w_tp, dtype=np.float64)

    Mk = np.zeros((4, 22, 22))
    for i, (l1, l2, l3) in enumerate(AGG_PATHS):
        a0, a1 = FEAT_SLICES[l1]
        b0, b1 = SH_SLICES[l2]
        c0, c1 = FEAT_SLICES[l3]
        alpha = math.sqrt((2 * l3 + 1) / AGG_NPATH[l3])
        cg = _CG_T[(l1, l2, l3)]
        for k in range(4):
            Mk[k, a0:a1, c0:c1] += (w_agg[i] * alpha) * np.einsum(
                'b,abc->ac', _SH[k, b0:b1], cg)

    # one packed lhsT per shift-source u (F00/F01/F10/F11): variant v's
    # aggregate matrix sits at columns [32v, 32v+22) so a single K=22
    # matmul per u accumulates all 4 variants' ctx into 32-aligned PSUM
    # strips (fp32r matmuls require dst partition base 0).
    per_u = [
        [Mk[0] + Mk[1] + Mk[2] + Mk[3], Mk[0] + Mk[2], Mk[0] + Mk[1], Mk[0]],
        [None, Mk[1] + Mk[3], None, Mk[1]],   # F01
        [None, None, Mk[2] + Mk[3], Mk[2]],   # F10
        [None, None, None, Mk[3]],            # F11
    ]
    s1w = np.zeros((4, 22, 128), dtype=np.float32)
    for u in range(4):
        for v in range(4):
            if per_u[u][v] is not None:
                s1w[u, :, 32 * v:32 * v + 22] = per_u[u][v]

    T = np.zeros((22, 22, 22))
    for i, (l1, l2, l3) in enumerate(TP_PATHS):
        a0, a1 = FEAT_SLICES[l1]
        b0, b1 = FEAT_SLICES[l2]
        c0, c1 = FEAT_SLICES[l3]
        alpha = math.sqrt((2 * l3 + 1) / TP_NPATH[l3])
        T[a0:a1, b0:b1, c0:c1] += (w_tp[i] * alpha) * _CG_T[(l1, l2, l3)]

    t_unf = np.ascontiguousarray(
        np.transpose(T, (1, 2, 0)).reshape(22, 484).astype(np.float32))
    return s1w, t_unf


# ---------------------------------------------------------------------------
# Device program (built & compiled once)
# ---------------------------------------------------------------------------
_PROG = None


def _build_program():
    global _PROG
    if _PROG is not None:
        return _PROG

    nc = bacc.Bacc("TRN2", target_bir_lowering=False, debug=False,
                   num_devices=N_CORES)

    f32r = mybir.dt.float32r

    featT_d = nc.dram_tensor("featT", [22, (ROWS_PER_CORE + 1) * WP], f32,
                             kind="ExternalInput").ap()
    fpm_d = nc.dram_tensor("feat_pm", [LR_PX_PER_CORE, 22], f32,
                           kind="ExternalInput").ap()
    s1w_d = nc.dram_tensor("s1w", [4, 22, 128], f32, kind="ExternalInput").ap()
    tunf_d = nc.dram_tensor("tunf", [22, 484], f32, kind="ExternalInput").ap()
    out_d = nc.dram_tensor("out", [ROWS_PER_CORE * R * W * R, 22], f32,
                           kind="ExternalOutput").ap()
    out3 = out_d.rearrange("(y x) c -> y x c", x=W * R)

    # stage-1: one packed matmul per shift-source u; shift in {0,1,WP,WP+1}
    S1_SHIFTS = [0, 1, WP, WP + 1]

    with tile.TileContext(nc) as tc:
        with (
            tc.tile_pool(name="const", bufs=1) as cpool,
            tc.tile_pool(name="work", bufs=4) as wpool,
            tc.tile_pool(name="psum_ctx", bufs=2, space="PSUM") as ctx_pool,
            tc.tile_pool(name="psum_g2", bufs=3, space="PSUM") as g2_pool,
        ):
            # --- constants / whole input, loaded once (f32r for the PE) ---
            featT_f = cpool.tile([22, ROWS_PER_CORE + 1, WP], f32)
            nc.sync.dma_start(
                out=featT_f[:, :, :],
                in_=featT_d.rearrange("c (r x) -> c r x", x=WP))
            featT_s = cpool.tile([22, ROWS_PER_CORE + 1, WP], f32r)
            nc.vector.tensor_copy(out=featT_s[:, :, :], in_=featT_f[:, :, :])

            s1w_f = cpool.tile([22, 4, 128], f32)
            nc.sync.dma_start(out=s1w_f[:, :, :],
                              in_=s1w_d.rearrange("u a c -> a u c"))
            s1w_s = cpool.tile([22, 4, 128], f32r)
            nc.vector.tensor_copy(out=s1w_s[:, :, :], in_=s1w_f[:, :, :])

            tunf_f = cpool.tile([22, 484], f32)
            nc.sync.dma_start(out=tunf_f[:, :], in_=tunf_d[:, :])
            tunf_s = cpool.tile([22, 484], f32r)
            nc.any.tensor_copy(out=tunf_s[:, :], in_=tunf_f[:, :])

            for t in range(N_TILES):
                # pixel-major F00 for the whole tile + its (c,a)-replicated
                # pattern (shared by all 4 variants' product scans) — emitted
                # first so DVE/GpSimd have work while PE runs stage 1
                fpm = wpool.tile([128, N_CHUNKS, 22], f32, tag="fpm")
                nc.sync.dma_start(
                    out=fpm[:, :, :],
                    in_=fpm_d[512 * t:512 * (t + 1), :]
                        .rearrange("(k p) c -> p k c", p=128))
                fpm_rep = wpool.tile([128, N_CHUNKS, 484], f32, tag="fpm_rep")
                nc.gpsimd.tensor_copy(
                    out=fpm_rep[:, :, :].rearrange("p k (c a) -> p k c a", a=22),
                    in_=fpm.unsqueeze(2).broadcast_to([128, N_CHUNKS, 22, 22]))

                # ---- stage 1: all 4 variants' ctx via 4 packed matmuls
                # into one [128, 512] PSUM tile (32-aligned variant strips) ----
                ctx_s = wpool.tile([22, 4, TILE_PX], f32r, tag="ctx_s")
                ctx_ps = ctx_pool.tile([128, TILE_PX], f32, tag="ctx_ps")
                for u, shift in enumerate(S1_SHIFTS):
                    dr, dx = divmod(shift, WP)
                    rhs = featT_s[:, 2 * t + dr: 2 * t + dr + 2, dx: dx + W]
                    nc.tensor.matmul(
                        ctx_ps[:, :],
                        lhsT=s1w_s[:, u, :],
                        rhs=rhs,
                        start=(u == 0), stop=(u == 3),
                    )
                for v in range(4):
                    nc.any.tensor_copy(out=ctx_s[:, v, :],
                                       in_=ctx_ps[32 * v:32 * v + 22, :])

                # ---- stage 2 per (variant, chunk-pair): G2 matmuls + fused
                # product/prefix-scan; segment sums via strided differences ----
                out_v = []
                for v in range(4):
                    ov = wpool.tile([128, N_CHUNKS, 22], f32, tag=f"ov{v}")
                    out_v.append(ov)
                    for h in range(2):
                        g2 = g2_pool.tile([128, 2, 512], f32, tag="g2")
                        for j in range(2):
                            chunk = 2 * h + j
                            lhsT = ctx_s[:, v, 128 * chunk:128 * chunk + 128]
                            nc.tensor.matmul(
                                g2[:, j, 0:484], lhsT=lhsT,
                                rhs=tunf_s[:, :],
                                start=True, stop=True,
                            )
                        scan = wpool.tile([128, 2, 484], f32, tag="scan",
                                          bufs=6)
                        nc.vector._custom_dve(
                            MUL_SCAN, out=scan[:, :, :],
                            in0=g2[:, :, 0:484],
                            in1=fpm_rep[:, 2 * h:2 * h + 2, :])
                        # prefix -> per-c sums: seg k sum = P[22k+21]-P[22k-1]
                        p3 = scan.rearrange("p k (c a) -> p (k c) a", a=22)
                        ov44 = ov[:, 2 * h:2 * h + 2, :].rearrange(
                            "p k c -> p (k c)")
                        nc.gpsimd.tensor_tensor(
                            out=ov44[:, 1:44], in0=p3[:, 1:44, 21],
                            in1=p3[:, 0:43, 21], op=mybir.AluOpType.subtract)
                        nc.gpsimd.tensor_copy(
                            out=ov44[:, 0:1], in_=p3[:, 0:1, 21])

                # ---- epilogue: residual add + 4x4 block replication ----
                bt = wpool.tile([128, N_CHUNKS, 4, 22], f32, tag="bt")
                bb = wpool.tile([128, N_CHUNKS, 4, 22], f32, tag="bb")
                res3 = fpm.unsqueeze(2).broadcast_to([128, N_CHUNKS, 3, 22])
                nc.gpsimd.tensor_tensor(
                    out=bt[:, :, 0:3, :], op=mybir.AluOpType.add,
                    in0=out_v[0].unsqueeze(2).broadcast_to([128, N_CHUNKS, 3, 22]),
                    in1=res3)
                nc.gpsimd.tensor_tensor(
                    out=bt[:, :, 3, :], op=mybir.AluOpType.add,
                    in0=out_v[1][:, :, :], in1=fpm[:, :, :])
                nc.gpsimd.tensor_tensor(
                    out=bb[:, :, 0:3, :], op=mybir.AluOpType.add,
                    in0=out_v[2].unsqueeze(2).broadcast_to([128, N_CHUNKS, 3, 22]),
                    in1=res3)
                nc.vector.tensor_tensor(
                    out=bb[:, :, 3, :], op=mybir.AluOpType.add,
                    in0=out_v[3][:, :, :], in1=fpm[:, :, :])

                # ---- stores: per chunk, one DMA for the 3 identical top
                # rows (source re-read) + one for the bottom row ----
                for chunk in range(N_CHUNKS):
                    lr = 2 * t + chunk // 2
                    xb = 512 * (chunk % 2)
                    nc.sync.dma_start(
                        out=out3[4 * lr:4 * lr + 3, xb:xb + 512, :]
                            .rearrange("y (p s) c -> p y s c", s=4),
                        in_=bt[:, chunk, :, :].unsqueeze(1)
                            .broadcast_to([128, 3, 4, 22]))
                    nc.scalar.dma_start(
                        out=out3[4 * lr + 3, xb:xb + 512, :]
                            .rearrange("(p s) c -> p s c", s=4),
                        in_=bb[:, chunk, :, :])

    nc.compile()
    _PROG = nc
    return nc


# ---------------------------------------------------------------------------
# Host wrapper
# ---------------------------------------------------------------------------
def _make_in_maps(f4, f6, w_agg, w_tp):
    f4 = np.asarray(f4, dtype=np.float32)
    f6 = np.asarray(f6, dtype=np.float32)
    feat = np.concatenate([f4, f6], axis=1).reshape(H, W, 22)
    featp = np.concatenate([feat, feat[:, -1:, :]], axis=1)  # [H, 257, 22]
    s1w, t_unf = _fold_weights(w_agg, w_tp)

    in_maps = []
    for k in range(N_CORES):
        rows = np.clip(np.arange(ROWS_PER_CORE * k, ROWS_PER_CORE * (k + 1) + 1),
                       0, H - 1)
        featT_k = np.ascontiguousarray(
            featp[rows].transpose(2, 0, 1).reshape(22, (ROWS_PER_CORE + 1) * WP))
        fpm_k = np.ascontiguousarray(
            feat.reshape(H * W, 22)[ROWS_PER_CORE * W * k:
                                    ROWS_PER_CORE * W * (k + 1)])
        in_maps.append({
            "featT": featT_k,
            "feat_pm": fpm_k,
            "s1w": s1w,
            "tunf": t_unf,
        })
    return in_maps


def _run(f4, f6, img_h, img_w, w_agg, w_tp, trace=False, tmpdir=None):
    assert int(img_h) == H and int(img_w) == W
    nc = _build_program()
    in_maps = _make_in_maps(f4, f6, np.asarray(w_agg), np.asarray(w_tp))
    res = run_bass_kernel_spmd(nc, in_maps, list(range(N_CORES)),
                               trace=trace, tmpdir=tmpdir)
    out_full = np.concatenate([res.results[k]["out"] for k in range(N_CORES)],
                              axis=0)
    f4_out = out_full[:, :9]
    f6_out = out_full[:, 9:22]
    return (f4_out, f6_out, (H * R, W * R)), res


def kernel(f4, f6, img_h, img_w, w_agg, w_tp):
    out, _ = _run(f4, f6, img_h, img_w, w_agg, w_tp, trace=False)
    return out


def kernel_traced(f4, f6, img_h, img_w, w_agg, w_tp, tmpdir=None):
    """Like kernel() but with NTFF profiling; returns (out, exec_time_ns)."""
    _install_axon_ntff_hook()
    out, res = _run(f4, f6, img_h, img_w, w_agg, w_tp, trace=True,
                    tmpdir=tmpdir)
    return out, res.exec_time_ns


def _install_axon_ntff_hook():
    if "antenv.axon_hooks" in sys.modules:
        return
    mod = types.ModuleType("antenv.axon_hooks")
    holder = {}
    mod.set_axon_ntff_profile_hook = lambda h: holder.__setitem__("h", h)
    mod.get_axon_ntff_profile_hook = lambda: holder.get("h")
    sys.modules["antenv.axon_hooks"] = mod
    if "/root/.axon_site" not in sys.path:
        sys.path.insert(0, "/root/.axon_site")
    try:
        from trn_agent_boot.trn_boot import _ntff_profile_via_ctypes
        mod.set_axon_ntff_profile_hook(
            _ntff_profile_via_ctypes("/opt/axon/libaxon_pjrt.so"))
    except Exception:
        pass


# revision 43
# speedup vs baseline: 1.2975x; 1.2975x over previous
"""Trainium2 Bass kernel for EquivariantUpsampleConv (H=W=256, R=4, C=22).

Strategy: data-parallel over LR rows (32 per core, 8 cores), with a 1-row
halo. Exploits the x4 nearest-neighbour upsample structure: within each
4x4 HR block there are only 4 distinct outputs (interior / right-edge /
bottom-edge / corner), so all compute runs at LR resolution (65536 px)
instead of HR (1048576 px); the 4x4 replication happens in the output DMA.

Per-pixel math: ctx_v = sum_u F_u @ M_{u,v} (stage-1, fp32r matmuls over
shifted channel-major feature views); out_v[c] = sum_{a,b} F00[a] ctx_v[b]
T[a,b,c] + F00[c]. Stage-2 per 128-px chunk: one fp32r matmul with
lhsT=ctx chunk gives G2[px,(c,a)] = sum_b T[a,b,c] ctx[b]; a custom DVE
op (MUL_SCAN_ANT) computes the running prefix sum of G2*broadcast(F00)
in one pass, and per-c segment sums are recovered as strided differences
of the prefix (fp32, stable). The residual is added in fp32 by the
epilogue block-replication adds.
"""

import math
import re
import sys
import types

import numpy as np

if "/opt/trn_rl_repo" not in sys.path:
    sys.path.insert(0, "/opt/trn_rl_repo")

import concourse.bacc as bacc
import concourse.dve_ops as dve_ops
import concourse.mybir as mybir
from concourse import tile
from concourse.bass_utils import run_bass_kernel_spmd
from concourse.dve_ops import DveOp
from concourse.dve_spec import AluOp, Scan, Spec, Src0, Src1

# ---------------------------------------------------------------------------
# Problem constants (hardcoded per spec)
# ---------------------------------------------------------------------------
H = W = 256
R = 4
C_FEAT = 22
N_CORES = 8
ROWS_PER_CORE = H // N_CORES          # 32 LR rows
LR_PX_PER_CORE = ROWS_PER_CORE * W    # 8192
WP = W + 1                            # 257: x-padded row stride
N_TILES = 16                          # 2 LR rows (512 px) per tile
TILE_PX = 512
N_CHUNKS = 4                          # 128-px chunks per tile

FEAT_SLICES = {4: (0, 9), 6: (9, 22)}
SH_SLICES = {0: (0, 1), 2: (1, 6)}
AGG_PATHS = [(4, 0, 4), (4, 2, 4), (4, 2, 6), (6, 0, 6), (6, 2, 4), (6, 2, 6)]
TP_PATHS = [(4, 4, 4), (4, 4, 6), (4, 6, 4), (4, 6, 6),
            (6, 4, 4), (6, 4, 6), (6, 6, 4), (6, 6, 6)]
AGG_NPATH = {4: 3, 6: 3}
TP_NPATH = {4: 4, 6: 4}

f32 = mybir.dt.float32


# ---------------------------------------------------------------------------
# Custom DVE op: prefix-scan of Src0*Src1 along the free stream
# ---------------------------------------------------------------------------
def _make_mul_scan():
    for o in dve_ops.OPS:
        if o.name == "MUL_SCAN_ANT":
            return o

    def _ref(in0, in1, c0, c1, c2):
        sh = in0.shape
        a = np.asarray(in0, np.float32).reshape(sh[0], -1)
        b = np.asarray(in1, np.float32).reshape(sh[0], -1)
        return np.add.accumulate(a * b, axis=1, dtype=np.float32).reshape(sh)

    spec = Spec(body=Scan(AluOp.ADD, Src0 * Src1), reference=_ref)
    dve_ops._SUB_OPCODE_FOR_NAME["MUL_SCAN_ANT"] = (
        dve_ops._CUSTOM_DVE_ROW_BASE + len(dve_ops.OPS))
    op = DveOp("MUL_SCAN_ANT", spec, subdim=False, uops_sha={})
    for ver in ("v3", "v4"):
        try:
            op.compile(ver)
        except ValueError as e:
            m = re.search(r'="([0-9a-f]+)"', str(e))
            assert m, str(e)
            op.uops_sha[ver] = m.group(1)
            dve_ops._COMPILE_CACHE.pop((op.name, ver), None)
    op.compile("v3")
    dve_ops.OPS.append(op)
    dve_ops.CUSTOM_DVE_SPECS[op.name] = spec
    return op


MUL_SCAN = _make_mul_scan()


# ---------------------------------------------------------------------------
# Clebsch-Gordan / spherical-harmonic constants (pure math, float64)
# ---------------------------------------------------------------------------
def _cg(j1, m1, j2, m2, j3, m3):
    if m1 + m2 != m3:
        return 0.0
    f = math.factorial
    pre = (2 * j3 + 1) * f(j1 + j2 - j3) * f(j1 - j2 + j3) * f(-j1 + j2 + j3) / f(j1 + j2 + j3 + 1)
    pre *= f(j1 + m1) * f(j1 - m1) * f(j2 + m2) * f(j2 - m2) * f(j3 + m3) * f(j3 - m3)
    kmin = max(0, j2 - j3 - m1, j1 - j3 + m2)
    kmax = min(j1 + j2 - j3, j1 - m1, j2 + m2)
    s = 0.0
    for k in range(kmin, kmax + 1):
        s += (-1) ** k / (f(k) * f(j1 + j2 - j3 - k) * f(j1 - m1 - k) * f(j2 + m2 - k)
                          * f(j3 - j2 + m1 + k) * f(j3 - j1 - m2 + k))
    return math.sqrt(pre) * s


def _q(l):
    q = np.zeros((2 * l + 1, 2 * l + 1), dtype=np.complex128)
    q[l, l] = 1.0
    s2 = 1.0 / math.sqrt(2.0)
    for m in range(1, l + 1):
        q[l + m, l + m] = (-1) ** m * s2
        q[l + m, l - m] = s2
        q[l - m, l - m] = 1j * s2
        q[l - m, l + m] = -1j * (-1) ** m * s2
    return ((-1j) ** l) * q


def _w3j_real(l1, l2, l3):
    w = np.zeros((2 * l1 + 1, 2 * l2 + 1, 2 * l3 + 1))
    for m1 in range(-l1, l1 + 1):
        for m2 in range(-l2, l2 + 1):
            m3 = -(m1 + m2)
            if abs(m3) <= l3:
                w[l1 + m1, l2 + m2, l3 + m3] = ((-1) ** (l1 - l2 - m3)) / math.sqrt(2 * l3 + 1) * _cg(l1, m1, l2, m2, l3, -m3)
    c = np.einsum('ai,bj,ck,ijk->abc', _q(l1), _q(l2), _q(l3), w.astype(np.complex128))
    return np.real(c)


_CG_T = {p: _w3j_real(*p) for p in set(AGG_PATHS) | set(TP_PATHS)}


def _sh_kernel():
    s = 1.0 / math.sqrt(2.0)
    dirs = np.array([[-s, -s, 0.0], [s, -s, 0.0], [-s, s, 0.0], [s, s, 0.0]])
    x, y, z = dirs[:, 0], dirs[:, 1], dirs[:, 2]
    c2 = math.sqrt(15.0 / math.pi)
    return np.stack([
        np.full(4, 0.5 / math.sqrt(math.pi)),
        0.5 * c2 * x * y,
        0.5 * c2 * y * z,
        0.25 * math.sqrt(5.0 / math.pi) * (3 * z ** 2 - 1.0),
        0.5 * c2 * x * z,
        0.25 * c2 * (x ** 2 - y ** 2),
    ], axis=1)  # (4, 6)


_SH = _sh_kernel()


def _fold_weights(w_agg, w_tp):
    """Fold runtime path weights into device matrices.

    Returns:
      s1w [9, 22, 22] float32: stage-1 lhsT mats (a-in -> c-out), order
        [S_int, S_A, S_B, S_C, S_D, M0, M1, M2, M3].
      t_unf [22, 484] float32: T[a,b,c] unfolded, col = c*22 + a, row b.
    """
    w_agg = np.asarray(w_agg, dtype=np.float64)
    w_tp = np.asarray(w_tp, dtype=np.float64)

    Mk = np.zeros((4, 22, 22))
    for i, (l1, l2, l3) in enumerate(AGG_PATHS):
        a0, a1 = FEAT_SLICES[l1]
        b0, b1 = SH_SLICES[l2]
        c0, c1 = FEAT_SLICES[l3]
        alpha = math.sqrt((2 * l3 + 1) / AGG_NPATH[l3])
        cg = _CG_T[(l1, l2, l3)]
        for k in range(4):
            Mk[k, a0:a1, c0:c1] += (w_agg[i] * alpha) * np.einsum(
                'b,abc->ac', _SH[k, b0:b1], cg)

    # one packed lhsT per shift-source u (F00/F01/F10/F11): variant v's
    # aggregate matrix sits at columns [32v, 32v+22) so a single K=22
    # matmul per u accumulates all 4 variants' ctx into 32-aligned PSUM
    # strips (fp32r matmuls require dst partition base 0).
    per_u = [
        [Mk[0] + Mk[1] + Mk[2] + Mk[3], Mk[0] + Mk[2], Mk[0] + Mk[1], Mk[0]],
        [None, Mk[1] + Mk[3], None, Mk[1]],   # F01
        [None, None, Mk[2] + Mk[3], Mk[2]],   # F10
        [None, None, None, Mk[3]],            # F11
    ]
    s1w = np.zeros((4, 22, 128), dtype=np.float32)
    for u in range(4):
        for v in range(4):
            if per_u[u][v] is not None:
                s1w[u, :, 32 * v:32 * v + 22] = per_u[u][v]

    T = np.zeros((22, 22, 22))
    for i, (l1, l2, l3) in enumerate(TP_PATHS):
        a0, a1 = FEAT_SLICES[l1]
        b0, b1 = FEAT_SLICES[l2]
        c0, c1 = FEAT_SLICES[l3]
        alpha = math.sqrt((2 * l3 + 1) / TP_NPATH[l3])
        T[a0:a1, b0:b1, c0:c1] += (w_tp[i] * alpha) * _CG_T[(l1, l2, l3)]

    t_unf = np.ascontiguousarray(
        np.transpose(T, (1, 2, 0)).reshape(22, 484).astype(np.float32))
    return s1w, t_unf


# ---------------------------------------------------------------------------
# Device program (built & compiled once)
# ---------------------------------------------------------------------------
_PROG = None


def _build_program():
    global _PROG
    if _PROG is not None:
        return _PROG

    nc = bacc.Bacc("TRN2", target_bir_lowering=False, debug=False,
                   num_devices=N_CORES)

    f32r = mybir.dt.float32r

    featT_d = nc.dram_tensor("featT", [22, (ROWS_PER_CORE + 1) * WP], f32,
                             kind="ExternalInput").ap()
    fpm_d = nc.dram_tensor("feat_pm", [LR_PX_PER_CORE, 22], f32,
                           kind="ExternalInput").ap()
    s1w_d = nc.dram_tensor("s1w", [4, 22, 128], f32, kind="ExternalInput").ap()
    tunf_d = nc.dram_tensor("tunf", [22, 484], f32, kind="ExternalInput").ap()
    out_d = nc.dram_tensor("out", [ROWS_PER_CORE * R * W * R, 22], f32,
                           kind="ExternalOutput").ap()
    out3 = out_d.rearrange("(y x) c -> y x c", x=W * R)

    # stage-1: one packed matmul per shift-source u; shift in {0,1,WP,WP+1}
    S1_SHIFTS = [0, 1, WP, WP + 1]

    with tile.TileContext(nc) as tc:
        with (
            tc.tile_pool(name="const", bufs=1) as cpool,
            tc.tile_pool(name="work", bufs=4) as wpool,
            tc.tile_pool(name="psum_ctx", bufs=2, space="PSUM") as ctx_pool,
            tc.tile_pool(name="psum_g2", bufs=3, space="PSUM") as g2_pool,
        ):
            # --- constants / whole input, loaded once (f32r for the PE) ---
            featT_f = cpool.tile([22, ROWS_PER_CORE + 1, WP], f32)
            nc.sync.dma_start(
                out=featT_f[:, :, :],
                in_=featT_d.rearrange("c (r x) -> c r x", x=WP))
            featT_s = cpool.tile([22, ROWS_PER_CORE + 1, WP], f32r)
            nc.vector.tensor_copy(out=featT_s[:, :, :], in_=featT_f[:, :, :])

            s1w_f = cpool.tile([22, 4, 128], f32)
            nc.sync.dma_start(out=s1w_f[:, :, :],
                              in_=s1w_d.rearrange("u a c -> a u c"))
            s1w_s = cpool.tile([22, 4, 128], f32r)
            nc.vector.tensor_copy(out=s1w_s[:, :, :], in_=s1w_f[:, :, :])

            tunf_f = cpool.tile([22, 484], f32)
            nc.sync.dma_start(out=tunf_f[:, :], in_=tunf_d[:, :])
            tunf_s = cpool.tile([22, 484], f32r)
            nc.any.tensor_copy(out=tunf_s[:, :], in_=tunf_f[:, :])

            for t in range(N_TILES):
                # ---- stage 1: all 4 variants' ctx via 4 packed matmuls
                # into one [128, 512] PSUM tile (32-aligned variant strips) ----
                ctx_s = wpool.tile([22, 4, TILE_PX], f32r, tag="ctx_s")
                ctx_ps = ctx_pool.tile([128, TILE_PX], f32, tag="ctx_ps")
                for u, shift in enumerate(S1_SHIFTS):
                    dr, dx = divmod(shift, WP)
                    rhs = featT_s[:, 2 * t + dr: 2 * t + dr + 2, dx: dx + W]
                    nc.tensor.matmul(
                        ctx_ps[:, :],
                        lhsT=s1w_s[:, u, :],
                        rhs=rhs,
                        start=(u == 0), stop=(u == 3),
                    )
                for v in range(4):
                    nc.any.tensor_copy(out=ctx_s[:, v, :],
                                       in_=ctx_ps[32 * v:32 * v + 22, :])

                # pixel-major F00 for the whole tile + its (c,a)-replicated
                # pattern (shared by all 4 variants' product scans)
                fpm = wpool.tile([128, N_CHUNKS, 22], f32, tag="fpm")
                nc.sync.dma_start(
                    out=fpm[:, :, :],
                    in_=fpm_d[512 * t:512 * (t + 1), :]
                        .rearrange("(k p) c -> p k c", p=128))
                fpm_rep = wpool.tile([128, N_CHUNKS, 484], f32, tag="fpm_rep")
                nc.any.tensor_copy(
                    out=fpm_rep[:, :, :].rearrange("p k (c a) -> p k c a", a=22),
                    in_=fpm.unsqueeze(2).broadcast_to([128, N_CHUNKS, 22, 22]))

                # ---- stage 2 per (variant, chunk-pair): G2 matmuls + fused
                # product/prefix-scan; segment sums via strided differences ----
                out_v = []
                for v in range(4):
                    ov = wpool.tile([128, N_CHUNKS, 22], f32, tag=f"ov{v}")
                    out_v.append(ov)
                    for h in range(2):
                        g2 = g2_pool.tile([128, 2, 512], f32, tag="g2")
                        for j in range(2):
                            chunk = 2 * h + j
                            lhsT = ctx_s[:, v, 128 * chunk:128 * chunk + 128]
                            nc.tensor.matmul(
                                g2[:, j, 0:484], lhsT=lhsT,
                                rhs=tunf_s[:, :],
                                start=True, stop=True,
                            )
                        scan = wpool.tile([128, 2, 484], f32, tag="scan",
                                          bufs=6)
                        nc.vector._custom_dve(
                            MUL_SCAN, out=scan[:, :, :],
                            in0=g2[:, :, 0:484],
                            in1=fpm_rep[:, 2 * h:2 * h + 2, :])
                        # prefix -> per-c sums: seg k sum = P[22k+21]-P[22k-1]
                        p3 = scan.rearrange("p k (c a) -> p (k c) a", a=22)
                        ov44 = ov[:, 2 * h:2 * h + 2, :].rearrange(
                            "p k c -> p (k c)")
                        nc.gpsimd.tensor_tensor(
                            out=ov44[:, 1:44], in0=p3[:, 1:44, 21],
                            in1=p3[:, 0:43, 21], op=mybir.AluOpType.subtract)
                        nc.gpsimd.tensor_copy(
                            out=ov44[:, 0:1], in_=p3[:, 0:1, 21])

                # ---- epilogue: residual add + 4x4 block replication ----
                bt = wpool.tile([128, N_CHUNKS, 4, 22], f32, tag="bt")
                bb = wpool.tile([128, N_CHUNKS, 4, 22], f32, tag="bb")
                res3 = fpm.unsqueeze(2).broadcast_to([128, N_CHUNKS, 3, 22])
                nc.gpsimd.tensor_tensor(
                    out=bt[:, :, 0:3, :], op=mybir.AluOpType.add,
                    in0=out_v[0].unsqueeze(2).broadcast_to([128, N_CHUNKS, 3, 22]),
                    in1=res3)
                nc.gpsimd.tensor_tensor(
                    out=bt[:, :, 3, :], op=mybir.AluOpType.add,
                    in0=out_v[1][:, :, :], in1=fpm[:, :, :])
                nc.gpsimd.tensor_tensor(
                    out=bb[:, :, 0:3, :], op=mybir.AluOpType.add,
                    in0=out_v[2].unsqueeze(2).broadcast_to([128, N_CHUNKS, 3, 22]),
                    in1=res3)
                nc.vector.tensor_tensor(
                    out=bb[:, :, 3, :], op=mybir.AluOpType.add,
                    in0=out_v[3][:, :, :], in1=fpm[:, :, :])

                # ---- stores: per chunk, one DMA for the 3 identical top
                # rows (source re-read) + one for the bottom row ----
                for chunk in range(N_CHUNKS):
                    lr = 2 * t + chunk // 2
                    xb = 512 * (chunk % 2)
                    nc.sync.dma_start(
                        out=out3[4 * lr:4 * lr + 3, xb:xb + 512, :]
                            .rearrange("y (p s) c -> p y s c", s=4),
                        in_=bt[:, chunk, :, :].unsqueeze(1)
                            .broadcast_to([128, 3, 4, 22]))
                    nc.scalar.dma_start(
                        out=out3[4 * lr + 3, xb:xb + 512, :]
                            .rearrange("(p s) c -> p s c", s=4),
                        in_=bb[:, chunk, :, :])

    nc.compile()
    _PROG = nc
    return nc


# ---------------------------------------------------------------------------
# Host wrapper
# ---------------------------------------------------------------------------
def _make_in_maps(f4, f6, w_agg, w_tp):
    f4 = np.asarray(f4, dtype=np.float32)
    f6 = np.asarray(f6, dtype=np.float32)
    feat = np.concatenate([f4, f6], axis=1).reshape(H, W, 22)
    featp = np.concatenate([feat, feat[:, -1:, :]], axis=1)  # [H, 257, 22]
    s1w, t_unf = _fold_weights(w_agg, w_tp)

    in_maps = []
    for k in range(N_CORES):
        rows = np.clip(np.arange(ROWS_PER_CORE * k, ROWS_PER_CORE * (k + 1) + 1),
                       0, H - 1)
        featT_k = np.ascontiguousarray(
            featp[rows].transpose(2, 0, 1).reshape(22, (ROWS_PER_CORE + 1) * WP))
        fpm_k = np.ascontiguousarray(
            feat.reshape(H * W, 22)[ROWS_PER_CORE * W * k:
                                    ROWS_PER_CORE * W * (k + 1)])
        in_maps.append({
            "featT": featT_k,
            "feat_pm": fpm_k,
            "s1w": s1w,
            "tunf": t_unf,
        })
    return in_maps


def _run(f4, f6, img_h, img_w, w_agg, w_tp, trace=False, tmpdir=None):
    assert int(img_h) == H and int(img_w) == W
    nc = _build_program()
    in_maps = _make_in_maps(f4, f6, np.asarray(w_agg), np.asarray(w_tp))
    res = run_bass_kernel_spmd(nc, in_maps, list(range(N_CORES)),
                               trace=trace, tmpdir=tmpdir)
    out_full = np.concatenate([res.results[k]["out"] for k in range(N_CORES)],
                              axis=0)
    f4_out = out_full[:, :9]
    f6_out = out_full[:, 9:22]
    return (f4_out, f6_out, (H * R, W * R)), res


def kernel(f4, f6, img_h, img_w, w_agg, w_tp):
    out, _ = _run(f4, f6, img_h, img_w, w_agg, w_tp, trace=False)
    return out


def kernel_traced(f4, f6, img_h, img_w, w_agg, w_tp, tmpdir=None):
    """Like kernel() but with NTFF profiling; returns (out, exec_time_ns)."""
    _install_axon_ntff_hook()
    out, res = _run(f4, f6, img_h, img_w, w_agg, w_tp, trace=True,
                    tmpdir=tmpdir)
    return out, res.exec_time_ns


def _install_axon_ntff_hook():
    if "antenv.axon_hooks" in sys.modules:
        return
    mod = types.ModuleType("antenv.axon_hooks")
    holder = {}
    mod.set_axon_ntff_profile_hook = lambda h: holder.__setitem__("h", h)
    mod.get_axon_ntff_profile_hook = lambda: holder.get("h")
    sys.modules["antenv.axon_hooks"] = mod
    if "/root/.axon_site" not in sys.path:
        sys.path.insert(0, "/root/.axon_site")
    try:
        from trn_agent_boot.trn_boot import _ntff_profile_via_ctypes
        mod.set_axon_ntff_profile_hook(
            _ntff_profile_via_ctypes("/opt/axon/libaxon_pjrt.so"))
    except Exception:
        pass
